# revision 9
# baseline (speedup 1.0000x reference)
"""Trainium2 Bass kernel for NeighborMLPConvLayerLinear (gnn_message_passing).

Strategy (8 NeuronCores, SPMD):
  - Edges (E=1.6M) sharded by output segment: core c owns segments
    [c*6250, (c+1)*6250) = 200k edges (+4800 pad slots). Segments are uniform
    (row_splits = arange*32), so the segment reduction is a stride-32 sum.
  - Gather: SINGLE int16 index stream over 256B "pair tokens". Token t packs
    rows (2t, 2t+1) of the [x_in | in_features] table as
    [row_even (128B) | row_odd - row_even (128B)] (delta encoding).
    dma_gather (SBUF-source, transpose=True) delivers token channels on
    partitions: p 0:64 = even row [x|F], p 64:128 = delta. A per-edge 0/1
    parity mask mm (broadcast-DMA'd from DRAM) reconstructs the edge's row on
    DVE: gm = g[0:64] + mm * g[64:128]. Token 0 = zeros for pad slots (mm=0).
    This halves Q7 SWDGE descriptor generation vs a dual lo/hi stream.
  - MLP: p1 = W1a^T gm + W1b^T x_out[seg] (stride-0 broadcast rhs) on PE;
    h = GELU(p1 + b1) on Scalar; p2 = W2aug^T [h; 1] on PE;
    eo = p2 * F_gm; segment sums via grouped tensor_reduce on DVE.
"""
import sys

sys.path.insert(0, "/opt/trn_rl_repo")

import numpy as np
import ml_dtypes

from concourse import bacc, bass, mybir, tile
from concourse import bass_utils

BF16 = mybir.dt.bfloat16
F32 = mybir.dt.float32
I16 = mybir.dt.int16

N = 50000
M = 50000
DEG = 32
C_IN = 32
HID = 64
C_OUT = 32

NCORES = 8
SEG_PER_CORE = M // NCORES            # 6250
E_PER_CORE = SEG_PER_CORE * DEG       # 200000
CH = 8192                             # edges per gather chunk
NCHUNK = 25                           # 204800 slots
SLOTS = NCHUNK * CH
SEG_PAD = SLOTS // DEG                # 6400 segments incl. padding
SEG_PER_CHUNK = CH // DEG             # 256
PSUM_CH = 1024                        # edges per psum tile
KSUB = CH // PSUM_CH                  # 8

NTOK = 25001                          # zero token + 25000 row pairs
RANKS = (NTOK + 127) // 128           # 196
TOK = RANKS * 128                     # 25088 token slots

_NC_CACHE = {}


def build_nc():
    if "nc" in _NC_CACHE:
        return _NC_CACHE["nc"]
    nc = bacc.Bacc("TRN2", target_bir_lowering=False, debug=False,
                   num_devices=NCORES)

    tbl = nc.dram_tensor("tbl", [128, RANKS * 128], BF16, kind="ExternalInput").ap()
    idx = nc.dram_tensor("idx", [NCHUNK, 128, CH // 16], I16, kind="ExternalInput").ap()
    mmd = nc.dram_tensor("mmd", [NCHUNK, CH], BF16, kind="ExternalInput").ap()
    xo = nc.dram_tensor("xo", [C_IN, SEG_PAD], BF16, kind="ExternalInput").ap()
    wx = nc.dram_tensor("wx", [64, HID], BF16, kind="ExternalInput").ap()
    w1b = nc.dram_tensor("w1b", [C_IN, HID], BF16, kind="ExternalInput").ap()
    w2 = nc.dram_tensor("w2", [HID + 1, C_OUT], BF16, kind="ExternalInput").ap()
    b1 = nc.dram_tensor("b1", [HID, 1], F32, kind="ExternalInput").ap()
    out = nc.dram_tensor("out", [C_OUT, SEG_PAD], F32, kind="ExternalOutput").ap()

    with tile.TileContext(nc) as tc:
        with (
            tc.tile_pool(name="tbl", bufs=1) as tblp,
            tc.tile_pool(name="w", bufs=1) as wp,
            tc.tile_pool(name="idx", bufs=2) as idxp,
            tc.tile_pool(name="mm", bufs=2) as mmp,
            tc.tile_pool(name="tmp", bufs=2) as tmpp,
            tc.tile_pool(name="g", bufs=2) as gp,
            tc.tile_pool(name="h", bufs=1) as hp,
            tc.tile_pool(name="eo", bufs=2) as eop,
            tc.tile_pool(name="red", bufs=2) as redp,
            tc.tile_pool(name="ps1", bufs=2, space="PSUM") as ps1,
            tc.tile_pool(name="ps2", bufs=2, space="PSUM") as ps2,
        ):
            sb_tbl = tblp.tile([128, RANKS * 128], BF16, tag="tbl")
            nc.sync.dma_start(out=sb_tbl[:], in_=tbl[:])

            sb_xo = wp.tile([C_IN, SEG_PAD], BF16, tag="xo")
            nc.sync.dma_start(out=sb_xo[:], in_=xo[:])
            sb_wx = wp.tile([64, HID], BF16, tag="wx")
            nc.sync.dma_start(out=sb_wx[:], in_=wx[:])
            sb_w1b = wp.tile([C_IN, HID], BF16, tag="w1b")
            nc.sync.dma_start(out=sb_w1b[:], in_=w1b[:])
            sb_w2 = wp.tile([HID + 1, C_OUT], BF16, tag="w2")
            nc.sync.dma_start(out=sb_w2[:], in_=w2[:])
            sb_b1 = wp.tile([HID, 1], F32, tag="b1")
            nc.sync.dma_start(out=sb_b1[:], in_=b1[:])

            # h staging: [HID+1, 2*PSUM_CH]; row HID stays 1.0 (bias-via-matmul)
            h_all = hp.tile([HID + 1, 2 * PSUM_CH], BF16, tag="h")
            nc.vector.memset(h_all[HID:HID + 1, :], 1.0)

            for t in range(NCHUNK):
                it = idxp.tile([128, CH // 16], I16, tag="i")
                nc.sync.dma_start(out=it[:], in_=idx[t])
                mm = mmp.tile([64, CH], BF16, tag="mm")
                nc.sync.dma_start(
                    out=mm[:], in_=mmd[t].unsqueeze(0).to_broadcast([64, CH]))

                g = gp.tile([128, CH], BF16, tag="g")
                nc.gpsimd.dma_gather(
                    out_ap=g[:].unsqueeze(1), in_ap=sb_tbl[:], idxs_ap=it[:],
                    num_idxs=CH, num_idxs_reg=CH, elem_size=128, transpose=True,
                    sbuf_tokens_per_rank=128, sbuf_free_dim_per_rank=256,
                    single_packet=False,
                )
                # row select: g[0:64] += parity * g_delta. The delta half is
                # DMA-shifted to base partition 0 (DVE lanes are
                # partition-locked; SBUF+SBUF DVE ops need equal bases).
                tmp = tmpp.tile([64, CH], BF16, tag="tmp")
                nc.sync.dma_start(out=tmp[:], in_=g[64:128, :])
                nc.vector.tensor_tensor(out=tmp[:], in0=tmp[:], in1=mm[:],
                                        op=mybir.AluOpType.mult)
                nc.vector.tensor_tensor(out=g[0:64, :], in0=g[0:64, :], in1=tmp[:],
                                        op=mybir.AluOpType.add)
                gm = g

                red = redp.tile([C_OUT, SEG_PER_CHUNK], F32, tag="red")
                for k in range(KSUB):
                    e0 = k * PSUM_CH
                    p1 = ps1.tile([HID, PSUM_CH], F32, tag="p1")
                    for j in range(PSUM_CH // 512):
                        c0 = e0 + j * 512
                        s0 = (t * CH + c0) // DEG  # first segment of this 512-block
                        nc.tensor.matmul(out=p1[:, j * 512:(j + 1) * 512],
                                         lhsT=sb_wx[:], rhs=gm[0:64, c0:c0 + 512],
                                         start=True, stop=False)
                        xo_b = sb_xo[:, s0:s0 + 16].unsqueeze(2).to_broadcast(
                            [C_IN, 16, DEG])
                        nc.tensor.matmul(out=p1[:, j * 512:(j + 1) * 512],
                                         lhsT=sb_w1b[:], rhs=xo_b,
                                         start=False, stop=True)
                    hs = h_all[:, (k % 2) * PSUM_CH:(k % 2 + 1) * PSUM_CH]
                    nc.scalar.activation(hs[0:HID, :], p1[:],
                                         mybir.ActivationFunctionType.Gelu,
                                         bias=sb_b1[:], scale=1.0)
                    p2 = ps2.tile([C_OUT, PSUM_CH], F32, tag="p2")
                    for j in range(PSUM_CH // 512):
                        nc.tensor.matmul(out=p2[:, j * 512:(j + 1) * 512],
                                         lhsT=sb_w2[:],
                                         rhs=hs[:, j * 512:(j + 1) * 512],
                                         start=True, stop=True)
                    eo = eop.tile([C_OUT, PSUM_CH], BF16, tag="eo")
                    nc.vector.tensor_tensor(out=eo[:], in0=p2[:],
                                            in1=gm[C_IN:64, e0:e0 + PSUM_CH],
                                            op=mybir.AluOpType.mult)
                    nc.vector.tensor_reduce(
                        out=red[:, k * (PSUM_CH // DEG):(k + 1) * (PSUM_CH // DEG)],
                        in_=eo[:].rearrange("p (s e) -> p s e", e=DEG),
                        axis=mybir.AxisListType.X, op=mybir.AluOpType.add)
                nc.sync.dma_start(
                    out=out[:, t * SEG_PER_CHUNK:(t + 1) * SEG_PER_CHUNK],
                    in_=red[:])
    nc.compile()
    _NC_CACHE["nc"] = nc
    return nc


def _wrap(a):
    """slot i -> partition i%16, col i//16; replicated over 8 groups."""
    w = a.reshape(NCHUNK, CH // 16, 16).transpose(0, 2, 1)  # [NCHUNK,16,CH/16]
    return np.tile(w, (1, 8, 1)).copy()                     # [NCHUNK,128,CH/16]


def prep_in_maps(x_in, x_out, in_features, neighbors_index, neighbors_row_splits,
                 W1, b1, W2, b2):
    x_in = np.asarray(x_in, np.float32)
    x_out = np.asarray(x_out, np.float32)
    in_features = np.asarray(in_features, np.float32)
    idx = np.asarray(neighbors_index, np.int64)
    W1 = np.asarray(W1, np.float32)
    b1v = np.asarray(b1, np.float32)
    W2 = np.asarray(W2, np.float32)
    b2v = np.asarray(b2, np.float32)

    # pair-token table: token 0 = zeros; token t>=1 = rows (2t-2, 2t-1) as
    # [row_even | row_odd - row_even], row = [x_in | in_features] bf16.
    rows = np.empty((N, 64), np.float32)
    rows[:, 0:C_IN] = x_in
    rows[:, C_IN:64] = in_features
    rows16 = rows.astype(ml_dtypes.bfloat16)
    toks = np.zeros((TOK, 128), dtype=ml_dtypes.bfloat16)
    toks[1:N // 2 + 1, 0:64] = rows16[0::2]
    toks[1:N // 2 + 1, 64:128] = (
        rows16[1::2].astype(np.float32) - rows16[0::2].astype(np.float32)
    ).astype(ml_dtypes.bfloat16)
    tbl = toks.reshape(RANKS, 128, 128).transpose(1, 0, 2).reshape(
        128, RANKS * 128).copy()

    wx = np.zeros((64, HID), dtype=ml_dtypes.bfloat16)
    wx[0:C_IN] = W1[0:C_IN].astype(ml_dtypes.bfloat16)
    w1b = W1[C_IN:].astype(ml_dtypes.bfloat16)
    w2aug = np.zeros((HID + 1, C_OUT), dtype=ml_dtypes.bfloat16)
    w2aug[0:HID] = (W2 / DEG).astype(ml_dtypes.bfloat16)
    w2aug[HID] = (b2v / DEG).astype(ml_dtypes.bfloat16)
    b1c = b1v.reshape(HID, 1).copy()

    in_maps = []
    pad = SLOTS - E_PER_CORE
    for c in range(NCORES):
        v = idx[c * E_PER_CORE:(c + 1) * E_PER_CORE]
        tok = np.concatenate([(v >> 1) + 1, np.zeros(pad, np.int64)]).astype(np.int16)
        par = np.concatenate([(v & 1).astype(np.float32), np.zeros(pad, np.float32)])
        mmc = par.astype(ml_dtypes.bfloat16).reshape(NCHUNK, CH)
        xoc = np.zeros((C_IN, SEG_PAD), dtype=ml_dtypes.bfloat16)
        xoc[:, :SEG_PER_CORE] = x_out[
            c * SEG_PER_CORE:(c + 1) * SEG_PER_CORE].T.astype(ml_dtypes.bfloat16)
        in_maps.append({
            "tbl": tbl, "idx": _wrap(tok), "mmd": mmc, "xo": xoc,
            "wx": wx, "w1b": w1b, "w2": w2aug, "b1": b1c,
        })
    return in_maps


def kernel(**inputs):
    in_maps = prep_in_maps(**inputs)
    global _LAST_IN_MAPS
    _LAST_IN_MAPS = in_maps
    nc = build_nc()
    res = bass_utils.run_bass_kernel_spmd(nc, in_maps, list(range(NCORES))).results
    out = np.empty((M, C_OUT), np.float32)
    for c in range(NCORES):
        out[c * SEG_PER_CORE:(c + 1) * SEG_PER_CORE] = \
            res[c]["out"][:, :SEG_PER_CORE].T
    return out


# revision 11
# speedup vs baseline: 1.1997x; 1.1997x over previous
"""Trainium2 Bass kernel for NeighborMLPConvLayerLinear (gnn_message_passing).

Strategy (8 NeuronCores, SPMD):
  - Edges (E=1.6M) sharded by output segment: core c owns segments
    [c*6250, (c+1)*6250) = 200k edges (+4800 pad slots). Segments are uniform
    (row_splits = arange*32), so the segment reduction is a stride-32 sum.
  - Gather: SINGLE int16 index stream over 256B "pair tokens". Token t packs
    rows (2t, 2t+1) of the [x_in | in_features] table as
    [row_even (128B) | row_odd - row_even (128B)] (delta encoding).
    dma_gather (SBUF-source, transpose=True) delivers token channels on
    partitions: p 0:64 = even row [x|F], p 64:128 = delta. A per-edge 0/1
    parity mask mm (broadcast-DMA'd from DRAM) reconstructs the edge's row on
    DVE: gm = g[0:64] + mm * g[64:128]. Token 0 = zeros for pad slots (mm=0).
    This halves Q7 SWDGE descriptor generation vs a dual lo/hi stream.
  - MLP: p1 = W1a^T gm + W1b^T x_out[seg] (stride-0 broadcast rhs) on PE;
    h = GELU(p1 + b1) on Scalar; p2 = W2aug^T [h; 1] on PE;
    eo = p2 * F_gm; segment sums via grouped tensor_reduce on DVE.
"""
import sys

sys.path.insert(0, "/opt/trn_rl_repo")

import numpy as np
import ml_dtypes

from concourse import bacc, bass, mybir, tile
from concourse import bass_utils

BF16 = mybir.dt.bfloat16
F32 = mybir.dt.float32
I16 = mybir.dt.int16

N = 50000
M = 50000
DEG = 32
C_IN = 32
HID = 64
C_OUT = 32

NCORES = 8
SEG_PER_CORE = M // NCORES            # 6250
E_PER_CORE = SEG_PER_CORE * DEG       # 200000
CH = 8192                             # edges per gather chunk
NCHUNK = 25                           # 204800 slots
SLOTS = NCHUNK * CH
SEG_PAD = SLOTS // DEG                # 6400 segments incl. padding
SEG_PER_CHUNK = CH // DEG             # 256
PSUM_CH = 1024                        # edges per psum tile
KSUB = CH // PSUM_CH                  # 8

NTOK = 25001                          # zero token + 25000 row pairs
RANKS = (NTOK + 127) // 128           # 196
TOK = RANKS * 128                     # 25088 token slots

_NC_CACHE = {}


def build_nc():
    if "nc" in _NC_CACHE:
        return _NC_CACHE["nc"]
    nc = bacc.Bacc("TRN2", target_bir_lowering=False, debug=False,
                   num_devices=NCORES)

    tbl = nc.dram_tensor("tbl", [128, RANKS * 128], BF16, kind="ExternalInput").ap()
    idx = nc.dram_tensor("idx", [NCHUNK, 128, CH // 16], I16, kind="ExternalInput").ap()
    mmd = nc.dram_tensor("mmd", [NCHUNK, CH], BF16, kind="ExternalInput").ap()
    xo = nc.dram_tensor("xo", [C_IN, SEG_PAD], BF16, kind="ExternalInput").ap()
    wx = nc.dram_tensor("wx", [64, HID], BF16, kind="ExternalInput").ap()
    w1b = nc.dram_tensor("w1b", [C_IN, HID], BF16, kind="ExternalInput").ap()
    w2 = nc.dram_tensor("w2", [HID + 1, C_OUT], BF16, kind="ExternalInput").ap()
    b1 = nc.dram_tensor("b1", [HID, 1], F32, kind="ExternalInput").ap()
    out = nc.dram_tensor("out", [C_OUT, SEG_PAD], F32, kind="ExternalOutput").ap()

    with tile.TileContext(nc) as tc:
        with (
            tc.tile_pool(name="tbl", bufs=1) as tblp,
            tc.tile_pool(name="w", bufs=1) as wp,
            tc.tile_pool(name="idx", bufs=2) as idxp,
            tc.tile_pool(name="mm", bufs=2) as mmp,
            tc.tile_pool(name="tmp", bufs=2) as tmpp,
            tc.tile_pool(name="g", bufs=2) as gp,
            tc.tile_pool(name="h", bufs=1) as hp,
            tc.tile_pool(name="eo", bufs=2) as eop,
            tc.tile_pool(name="red", bufs=2) as redp,
            tc.tile_pool(name="ps1", bufs=2, space="PSUM") as ps1,
            tc.tile_pool(name="ps2", bufs=2, space="PSUM") as ps2,
        ):
            sb_tbl = tblp.tile([128, RANKS * 128], BF16, tag="tbl")
            nc.sync.dma_start(out=sb_tbl[:], in_=tbl[:])

            sb_xo = wp.tile([C_IN, SEG_PAD], BF16, tag="xo")
            nc.sync.dma_start(out=sb_xo[:], in_=xo[:])
            sb_wx = wp.tile([64, HID], BF16, tag="wx")
            nc.sync.dma_start(out=sb_wx[:], in_=wx[:])
            sb_w1b = wp.tile([C_IN, HID], BF16, tag="w1b")
            nc.sync.dma_start(out=sb_w1b[:], in_=w1b[:])
            sb_w2 = wp.tile([HID + 1, C_OUT], BF16, tag="w2")
            nc.sync.dma_start(out=sb_w2[:], in_=w2[:])
            sb_b1 = wp.tile([HID, 1], F32, tag="b1")
            nc.sync.dma_start(out=sb_b1[:], in_=b1[:])

            # h staging: [HID+1, 2*PSUM_CH]; row HID stays 1.0 (bias-via-matmul)
            h_all = hp.tile([HID + 1, 2 * PSUM_CH], BF16, tag="h")
            nc.vector.memset(h_all[HID:HID + 1, :], 1.0)

            for t in range(NCHUNK):
                it = idxp.tile([128, CH // 16], I16, tag="i")
                nc.sync.dma_start(out=it[:], in_=idx[t])
                mm = mmp.tile([64, CH], BF16, tag="mm")
                nc.sync.dma_start(
                    out=mm[:], in_=mmd[t].unsqueeze(0).to_broadcast([64, CH]))

                g = gp.tile([128, CH], BF16, tag="g")
                GH = CH // 2
                for gi in range(2):
                    nc.gpsimd.dma_gather(
                        out_ap=g[:, gi * GH:(gi + 1) * GH].unsqueeze(1),
                        in_ap=sb_tbl[:],
                        idxs_ap=it[:, gi * (GH // 16):(gi + 1) * (GH // 16)],
                        num_idxs=GH, num_idxs_reg=GH, elem_size=128, transpose=True,
                        sbuf_tokens_per_rank=128, sbuf_free_dim_per_rank=256,
                        single_packet=False,
                    )
                # row select: g[0:64] += parity * g_delta. The delta half is
                # DMA-shifted to base partition 0 (DVE lanes are
                # partition-locked; SBUF+SBUF DVE ops need equal bases).
                tmp = tmpp.tile([64, CH], BF16, tag="tmp")
                nc.sync.dma_start(out=tmp[:], in_=g[64:128, :])
                nc.vector.tensor_tensor(out=tmp[:], in0=tmp[:], in1=mm[:],
                                        op=mybir.AluOpType.mult)
                nc.vector.tensor_tensor(out=g[0:64, :], in0=g[0:64, :], in1=tmp[:],
                                        op=mybir.AluOpType.add)
                gm = g

                red = redp.tile([C_OUT, SEG_PER_CHUNK], F32, tag="red")
                for k in range(KSUB):
                    e0 = k * PSUM_CH
                    p1 = ps1.tile([HID, PSUM_CH], F32, tag="p1")
                    for j in range(PSUM_CH // 512):
                        c0 = e0 + j * 512
                        s0 = (t * CH + c0) // DEG  # first segment of this 512-block
                        nc.tensor.matmul(out=p1[:, j * 512:(j + 1) * 512],
                                         lhsT=sb_wx[:], rhs=gm[0:64, c0:c0 + 512],
                                         start=True, stop=False)
                        xo_b = sb_xo[:, s0:s0 + 16].unsqueeze(2).to_broadcast(
                            [C_IN, 16, DEG])
                        nc.tensor.matmul(out=p1[:, j * 512:(j + 1) * 512],
                                         lhsT=sb_w1b[:], rhs=xo_b,
                                         start=False, stop=True)
                    hs = h_all[:, (k % 2) * PSUM_CH:(k % 2 + 1) * PSUM_CH]
                    nc.scalar.activation(hs[0:HID, :], p1[:],
                                         mybir.ActivationFunctionType.Gelu,
                                         bias=sb_b1[:], scale=1.0)
                    p2 = ps2.tile([C_OUT, PSUM_CH], F32, tag="p2")
                    for j in range(PSUM_CH // 512):
                        nc.tensor.matmul(out=p2[:, j * 512:(j + 1) * 512],
                                         lhsT=sb_w2[:],
                                         rhs=hs[:, j * 512:(j + 1) * 512],
                                         start=True, stop=True)
                    eo = eop.tile([C_OUT, PSUM_CH], BF16, tag="eo")
                    nc.vector.tensor_tensor(out=eo[:], in0=p2[:],
                                            in1=gm[C_IN:64, e0:e0 + PSUM_CH],
                                            op=mybir.AluOpType.mult)
                    nc.vector.tensor_reduce(
                        out=red[:, k * (PSUM_CH // DEG):(k + 1) * (PSUM_CH // DEG)],
                        in_=eo[:].rearrange("p (s e) -> p s e", e=DEG),
                        axis=mybir.AxisListType.X, op=mybir.AluOpType.add)
                nc.sync.dma_start(
                    out=out[:, t * SEG_PER_CHUNK:(t + 1) * SEG_PER_CHUNK],
                    in_=red[:])
    nc.compile()
    _NC_CACHE["nc"] = nc
    return nc


def _wrap(a):
    """slot i -> partition i%16, col i//16; replicated over 8 groups."""
    w = a.reshape(NCHUNK, CH // 16, 16).transpose(0, 2, 1)  # [NCHUNK,16,CH/16]
    return np.tile(w, (1, 8, 1)).copy()                     # [NCHUNK,128,CH/16]


def prep_in_maps(x_in, x_out, in_features, neighbors_index, neighbors_row_splits,
                 W1, b1, W2, b2):
    x_in = np.asarray(x_in, np.float32)
    x_out = np.asarray(x_out, np.float32)
    in_features = np.asarray(in_features, np.float32)
    idx = np.asarray(neighbors_index, np.int64)
    W1 = np.asarray(W1, np.float32)
    b1v = np.asarray(b1, np.float32)
    W2 = np.asarray(W2, np.float32)
    b2v = np.asarray(b2, np.float32)

    # pair-token table: token 0 = zeros; token t>=1 = rows (2t-2, 2t-1) as
    # [row_even | row_odd - row_even], row = [x_in | in_features] bf16.
    rows = np.empty((N, 64), np.float32)
    rows[:, 0:C_IN] = x_in
    rows[:, C_IN:64] = in_features
    rows16 = rows.astype(ml_dtypes.bfloat16)
    toks = np.zeros((TOK, 128), dtype=ml_dtypes.bfloat16)
    toks[1:N // 2 + 1, 0:64] = rows16[0::2]
    toks[1:N // 2 + 1, 64:128] = (
        rows16[1::2].astype(np.float32) - rows16[0::2].astype(np.float32)
    ).astype(ml_dtypes.bfloat16)
    tbl = toks.reshape(RANKS, 128, 128).transpose(1, 0, 2).reshape(
        128, RANKS * 128).copy()

    wx = np.zeros((64, HID), dtype=ml_dtypes.bfloat16)
    wx[0:C_IN] = W1[0:C_IN].astype(ml_dtypes.bfloat16)
    w1b = W1[C_IN:].astype(ml_dtypes.bfloat16)
    w2aug = np.zeros((HID + 1, C_OUT), dtype=ml_dtypes.bfloat16)
    w2aug[0:HID] = (W2 / DEG).astype(ml_dtypes.bfloat16)
    w2aug[HID] = (b2v / DEG).astype(ml_dtypes.bfloat16)
    b1c = b1v.reshape(HID, 1).copy()

    in_maps = []
    pad = SLOTS - E_PER_CORE
    for c in range(NCORES):
        v = idx[c * E_PER_CORE:(c + 1) * E_PER_CORE]
        tok = np.concatenate([(v >> 1) + 1, np.zeros(pad, np.int64)]).astype(np.int16)
        par = np.concatenate([(v & 1).astype(np.float32), np.zeros(pad, np.float32)])
        mmc = par.astype(ml_dtypes.bfloat16).reshape(NCHUNK, CH)
        xoc = np.zeros((C_IN, SEG_PAD), dtype=ml_dtypes.bfloat16)
        xoc[:, :SEG_PER_CORE] = x_out[
            c * SEG_PER_CORE:(c + 1) * SEG_PER_CORE].T.astype(ml_dtypes.bfloat16)
        in_maps.append({
            "tbl": tbl, "idx": _wrap(tok), "mmd": mmc, "xo": xoc,
            "wx": wx, "w1b": w1b, "w2": w2aug, "b1": b1c,
        })
    return in_maps


def kernel(**inputs):
    in_maps = prep_in_maps(**inputs)
    global _LAST_IN_MAPS
    _LAST_IN_MAPS = in_maps
    nc = build_nc()
    res = bass_utils.run_bass_kernel_spmd(nc, in_maps, list(range(NCORES))).results
    out = np.empty((M, C_OUT), np.float32)
    for c in range(NCORES):
        out[c * SEG_PER_CORE:(c + 1) * SEG_PER_CORE] = \
            res[c]["out"][:, :SEG_PER_CORE].T
    return out
